# revision 2
# baseline (speedup 1.0000x reference)
"""DiffuseRouter kernel for 8 TRN2 NeuronCores.

Reference computation (enable_time=False, soft_time_routing=True):
    out[b, l, d] = (1/3) * sum_g sum_e expert_emb_g[e, b, l, d]
i.e. a uniform-weighted sum of 28 expert planes per batch element.

Sharding: pure data-parallel over batch B=8 -> one batch element per core.
Each core reads its 28 [256, 1280] f32 planes (36.7 MB), reduces them
on-chip, scales by 1/3, and writes its [256, 1280] output.  No collectives
needed (B == n_cores), which is strictly less traffic than expert-parallel
+ all-reduce.
"""

import numpy as np

import concourse.bacc as bacc
import concourse.tile as tile
from concourse import mybir
from concourse.bass_utils import run_bass_kernel_spmd

N_CORES = 8
E_TOTAL = 28  # 4 + 8 + 16 experts across the 3 granularity levels
L, D = 256, 1280
P = 128  # SBUF partitions
FD = (L // P) * D  # 2560 free-dim elements per partition
SCALE = 1.0 / 3.0

_NC_CACHE = None


def _build_nc():
    """Build the SPMD Bass program (identical on all 8 cores)."""
    nc = bacc.Bacc("TRN2", target_bir_lowering=False, debug=False)
    x = nc.dram_tensor("x", [E_TOTAL, L, D], mybir.dt.float32, kind="ExternalInput")
    out = nc.dram_tensor("out", [L, D], mybir.dt.float32, kind="ExternalOutput")

    # [E, 256, 1280] -> [E, 128, 2560]: partition p holds rows 2p, 2p+1
    # (contiguous 10240 B per partition -> fully linear 1.31 MB DMA per plane).
    x_t = x.ap().rearrange("e (p a) d -> e p (a d)", a=2)
    out_t = out.ap().rearrange("(p a) d -> p (a d)", a=2)

    with tile.TileContext(nc) as tc:
        with (
            tc.tile_pool(name="in", bufs=6) as pin,
            tc.tile_pool(name="acc", bufs=1) as pacc,
            tc.tile_pool(name="res", bufs=1) as pres,
        ):
            acc = None
            first = None
            for e in range(E_TOTAL):
                t = pin.tile([P, FD], mybir.dt.float32)
                nc.sync.dma_start(out=t[:], in_=x_t[e])
                if first is None:
                    first = t
                elif acc is None:
                    acc = pacc.tile([P, FD], mybir.dt.float32)
                    nc.vector.tensor_add(acc[:], first[:], t[:])
                else:
                    nc.vector.tensor_add(acc[:], acc[:], t[:])

            res = pres.tile([P, FD], mybir.dt.float32)
            nc.scalar.mul(res[:], acc[:], SCALE)
            nc.sync.dma_start(out=out_t, in_=res[:])
    nc.compile()
    return nc


def _get_nc():
    global _NC_CACHE
    if _NC_CACHE is None:
        _NC_CACHE = _build_nc()
    return _NC_CACHE


def _run(inputs, trace=False, trace_kwargs=None):
    e0 = np.asarray(inputs["expert_emb_0"], dtype=np.float32)
    e1 = np.asarray(inputs["expert_emb_1"], dtype=np.float32)
    e2 = np.asarray(inputs["expert_emb_2"], dtype=np.float32)
    B = e0.shape[1]
    assert B == N_CORES, f"expected B == {N_CORES}, got {B}"

    in_maps = []
    for b in range(B):
        xb = np.concatenate([e0[:, b], e1[:, b], e2[:, b]], axis=0)
        in_maps.append({"x": np.ascontiguousarray(xb)})

    kw = {}
    if trace:
        kw["trace"] = True
        if trace_kwargs:
            kw.update(trace_kwargs)
    res = run_bass_kernel_spmd(_get_nc(), in_maps, list(range(N_CORES)), **kw)
    out = np.stack([res.results[b]["out"] for b in range(B)], axis=0)
    return out.astype(np.float32, copy=False), res


def kernel(**inputs) -> np.ndarray:
    out, _ = _run(inputs, trace=False)
    return out


# revision 6
# speedup vs baseline: 1.1349x; 1.1349x over previous
"""DiffuseRouter kernel for 8 TRN2 NeuronCores.

Reference computation (enable_time=False, soft_time_routing=True):
    out[b, l, d] = (1/3) * sum_g sum_e expert_emb_g[e, b, l, d]
i.e. a uniform-weighted sum of 28 expert planes per batch element.

Sharding: pure data-parallel over batch B=8 -> one batch element per core.
Each core reads its 28 [256, 1280] f32 planes (36.7 MB), reduces them
on-chip, scales by 1/3, and writes its [256, 1280] output.  No collectives
needed (B == n_cores), which is strictly less traffic than expert-parallel
+ all-reduce.
"""

import numpy as np

import concourse.bacc as bacc
import concourse.tile as tile
from concourse import mybir
from concourse.alu_op_type import AluOpType
from concourse.bass_utils import run_bass_kernel_spmd

N_CORES = 8
E_TOTAL = 28  # 4 + 8 + 16 experts across the 3 granularity levels
L, D = 256, 1280
P = 128  # SBUF partitions
FD = (L // P) * D  # 2560 free-dim elements per partition
SCALE = 1.0 / 3.0

_NC_CACHE = None


def _build_nc():
    """Build the SPMD Bass program (identical on all 8 cores).

    Structure: stream the 28 expert planes as [128, 2560] tiles (1.31 MB
    linear DMAs) on the SP HWDGE ring; accumulate on DVE in two independent
    half-chains over the free dim (cols [0:1280) and [1280:2560)) with the
    1/3 scale folded into every add via scalar_tensor_tensor, so each half
    can be stored the moment its last add retires.  Stores go on the ACT
    HWDGE ring so they never queue behind input loads.
    """
    nc = bacc.Bacc(
        "TRN2", target_bir_lowering=False, debug=False, enable_partition_id=False
    )
    x = nc.dram_tensor("x", [E_TOTAL, L, D], mybir.dt.float32, kind="ExternalInput")
    out = nc.dram_tensor("out", [L, D], mybir.dt.float32, kind="ExternalOutput")

    # [E, 256, 1280] -> [E, 128, 2560]: partition p holds rows 2p, 2p+1
    # (contiguous 10240 B per partition -> fully linear 1.31 MB DMA per plane).
    x_t = x.ap().rearrange("e (p a) d -> e p (a d)", a=2)
    out_t = out.ap().rearrange("(p a) d -> p (a d)", a=2)

    H = FD // 2  # half of the free dim
    halves = [slice(0, H), slice(H, FD)]
    mult = AluOpType.mult
    add = AluOpType.add

    with tile.TileContext(nc) as tc:
        with (
            tc.tile_pool(name="in", bufs=6) as pin,
            tc.tile_pool(name="acc", bufs=2) as pacc,
        ):
            accs = [
                pacc.tile(
                    [P, H], mybir.dt.float32, name=f"acc{i}", tag=f"acc{i}"
                )
                for i in range(2)
            ]
            for e in range(E_TOTAL):
                t = pin.tile([P, FD], mybir.dt.float32)
                nc.sync.dma_start(out=t[:], in_=x_t[e])
                for acc, h in zip(accs, halves):
                    if e == 0:
                        # acc = t0 * 1/3 (tensor_scalar: 2x perf mode)
                        nc.vector.tensor_scalar_mul(acc[:], t[:, h], SCALE)
                    else:
                        # acc = (t_e * 1/3) + acc
                        nc.vector.scalar_tensor_tensor(
                            acc[:], t[:, h], SCALE, acc[:], mult, add
                        )
            for acc, h in zip(accs, halves):
                nc.scalar.dma_start(out=out_t[:, h], in_=acc[:])
    nc.compile()
    return nc


def _get_nc():
    global _NC_CACHE
    if _NC_CACHE is None:
        _NC_CACHE = _build_nc()
    return _NC_CACHE


def _run(inputs, trace=False, trace_kwargs=None):
    e0 = np.asarray(inputs["expert_emb_0"], dtype=np.float32)
    e1 = np.asarray(inputs["expert_emb_1"], dtype=np.float32)
    e2 = np.asarray(inputs["expert_emb_2"], dtype=np.float32)
    B = e0.shape[1]
    assert B == N_CORES, f"expected B == {N_CORES}, got {B}"

    in_maps = []
    for b in range(B):
        xb = np.concatenate([e0[:, b], e1[:, b], e2[:, b]], axis=0)
        in_maps.append({"x": np.ascontiguousarray(xb)})

    kw = {}
    if trace:
        kw["trace"] = True
        if trace_kwargs:
            kw.update(trace_kwargs)
    res = run_bass_kernel_spmd(_get_nc(), in_maps, list(range(N_CORES)), **kw)
    out = np.stack([res.results[b]["out"] for b in range(B)], axis=0)
    return out.astype(np.float32, copy=False), res


def kernel(**inputs) -> np.ndarray:
    out, _ = _run(inputs, trace=False)
    return out
